# revision 26
# baseline (speedup 1.0000x reference)
"""Trainium2 Bass kernel for the DPAAUser3D segment-reduce problem.

Computes, for x[B=2,C=8,D=H=W=128] and attentions[B,C,512,1]:
  onehot = one_hot(argmax_c x)                      (per-voxel channel argmax)
  adj    = avgpool_8x8x8(onehot)                    ([B,C,16,16,16], = counts/512)
  corr[b,c,D,H,W] = att[b,c,(D//16*8+H//16)*8+W//16] * adj[b,c,D%16,H%16,W%16]
  out1   = x * (1+corr)^2
  out2   = corr

Single-pass design: x is loaded once per core (data-parallel over D, 16
d-slices each) and stays resident in SBUF between the counting phase and the
output phase; all bulk IO is fp16 (~27 MiB per core HBM traffic).

Per batch, x is ONE SBUF mega-tile with partitions=(d_local, h//16) and
free=(c, h%16, w); the DRAM tensors use the matching packed layout so every
load/store is a fully-contiguous 2.1 MiB DMA. Elementwise work runs as
mega-ops over the whole [128, 16384] tile (amortizes the ~0.4us DVE
instruction overhead; fp16 + unit stride hits the DVE 2x packing mode).
Pooled counts: 8 accumulating PE matmuls against a 0/1 selector fold the
d-blocks AND the w%8 axis (rhs access pattern shifted by one w each call),
leaving one tiny DVE reduce per channel. A per-batch 8 KiB AllGather
distributes the count map; out1 is computed in place over the resident x.

fp16 correctness: the host computes the f32 argmax (reference semantics) and
nudges any non-argmax channel that collides with the channel max in fp16 down
one ulp, so the device equality compare reproduces one_hot(argmax(f32 x))
exactly and the pooled counts are exact integers (<=512, exact in fp16). The
remaining error is fp16 rounding on x and on each output store (~1e-3 rel).
"""

import sys

import numpy as np

try:
    import concourse.bass as bass
except ImportError:  # fresh grading dir: concourse lives in the repo checkout
    for p in ("/opt/trn_rl_repo", "/root/.axon_site/_ro/trn_rl_repo"):
        if p not in sys.path:
            sys.path.insert(0, p)
    import concourse.bass as bass

import concourse.bacc as bacc
import concourse.mybir as mybir
import concourse.tile as tile
from concourse import bass_utils

B, C, D, H, W = 2, 8, 128, 128, 128
POOL = 8          # pooling block edge
PATCH = 16        # fold patch edge
G = D // PATCH    # 8 patches per spatial dim
NCORES = 8
DL = D // NCORES  # 16 d-slices per core
PD = DL // POOL   # 2 pooled kd-blocks per core
FS = C * PATCH * PATCH  # 2048: free size of the per-batch pooled-count map
NH = 2            # C split into halves of 4 channels for load/store overlap
CH = C // NH
CF = PATCH * W    # 2048: per-channel free size

F32 = mybir.dt.float32
F16 = mybir.dt.float16

_CACHE = {}


def _build_nc():
    nc = bacc.Bacc("TRN2", target_bir_lowering=False, debug=False,
                   num_devices=NCORES)

    # packed layout: [b, half, dl, hb, c4, k, w] (partition = (dl, hb))
    xs = nc.dram_tensor("xs", [B, NH, DL, POOL, CH, PATCH, W], F16,
                        kind="ExternalInput").ap()
    # arep[b,c,p,wp] = att[b,c, (core*8 + p%8)*8 + wp] / 512   (p = dl*8+hb)
    arep = nc.dram_tensor("arep", [B, C, 128, G], F32,
                          kind="ExternalInput").ap()
    # sel[p, (p//64)*8 + p%8] = 1: contracts the two 8-d-slice blocks
    sel = nc.dram_tensor("sel", [128, 16], F16, kind="ExternalInput").ap()
    o1 = nc.dram_tensor("o1", [B, NH, DL, POOL, CH, PATCH, W], F16,
                        kind="ExternalOutput").ap()
    o2 = nc.dram_tensor("o2", [B, NH, DL, POOL, CH, PATCH, W], F16,
                        kind="ExternalOutput").ap()

    with tile.TileContext(nc) as tc:
        with (
            tc.tile_pool(name="big", bufs=1) as big,
            tc.tile_pool(name="psum", bufs=2, space="PSUM") as pp,
            tc.tile_pool(name="dram", bufs=1, space="DRAM") as dram,
        ):
            Sel = big.tile([128, 16], F16, name="Sel")
            Ar = big.tile([128, B, C, G], F32, name="Ar")
            X = {b: big.tile([128, C, CF], F16, name=f"x{b}") for b in range(B)}
            EQ = big.tile([128, C, CF], F16, name="EQ")
            CO = big.tile([128, C, CF], F16, name="CO")
            U2 = big.tile([128, CH, CF], F16, name="U2")  # one half at a time
            m1 = [big.tile([128, CF], F16, name=f"m1_{i}") for i in range(4)]
            m2 = [big.tile([128, CF], F16, name=f"m2_{i}") for i in range(2)]
            M = big.tile([128, CF], F16, name="M")
            # AdjR[p=(dl,rep8), (b,j,c,l)] = counts row kd=dl (8x part rep)
            AdjR = big.tile([128, B, PATCH, C, PATCH], F16, name="AdjR")
            Cnt = [big.tile([16, 2, C, PATCH], F16, name=f"cnt{b}")
                   for b in range(B)]

            nc.sync.dma_start(out=Sel, in_=sel)
            for b in range(B):
                nc.sync.dma_start(out=Ar[:, b], in_=arep[b].transpose([1, 0, 2]))
            for b in range(B):
                for h in range(NH):
                    xv = xs[b, h].rearrange("dl hb c k w -> (dl hb) (c k w)")
                    nc.sync.dma_start(out=X[b][:, h * CH:(h + 1) * CH], in_=xv)

            # row layout per (pd): (b, j, c, l) so BOTH batches ride one gather
            adj_in = dram.tile([PD, B, PATCH, C, PATCH], F16, name="adj_in")
            adj_gat = dram.tile([NCORES, PD, B, PATCH, C, PATCH], F16,
                                name="adj_gat", addr_space="Shared")

            def phase1(b):
                # channel max: pairwise tree (fp16 unit-stride -> DVE 2x mode)
                for i in range(4):
                    nc.vector.tensor_max(m1[i], X[b][:, 2 * i], X[b][:, 2 * i + 1])
                nc.vector.tensor_max(m2[0], m1[0], m1[1])
                nc.vector.tensor_max(m2[1], m1[2], m1[3])
                nc.vector.tensor_max(M, m2[0], m2[1])
                # one-hot in a single mega-op
                nc.vector.tensor_tensor(
                    EQ, X[b], M.unsqueeze(1).broadcast_to([128, C, CF]),
                    op=mybir.AluOpType.is_equal)
                # pooled counts: 8 accumulating matmuls fold d-blocks (via Sel
                # on partitions) and w%8 (via rhs offset j); one reduce folds k%8
                for c in range(C):
                    ps = pp.tile([16, PATCH * PATCH], F32, name="ps",
                                 tag=f"ps{c % 4}")
                    eqc = EQ[:, c].rearrange("p (k l w8) -> p (k l) w8",
                                             k=PATCH, l=PATCH, w8=POOL)
                    for j in range(POOL):
                        nc.tensor.matmul(ps, lhsT=Sel, rhs=eqc[:, :, j],
                                         start=(j == 0), stop=(j == POOL - 1))
                    pv = ps.rearrange("p (k2 kk l) -> p k2 l kk",
                                      k2=2, kk=POOL, l=PATCH)
                    # counts are integers <=512: exact in fp16
                    with nc.allow_low_precision(reason="integer counts"):
                        nc.vector.reduce_sum(Cnt[b][:, :, c], pv,
                                             axis=mybir.AxisListType.X)
                # ship counts: partition (pd,hb) + free (j2,c,l) lands on the
                # contiguous (j=hb*2+j2, c, l) rows of adj_in[:, b]
                FR = PATCH * C * PATCH  # 2048: per-(pd,b) row size
                cin = bass.AP(tensor=adj_in.tensor,
                              offset=adj_in.offset + b * FR,
                              ap=[[B * FR, PD], [2 * C * PATCH, POOL],
                                  [1, 2 * C * PATCH]])
                nc.gpsimd.dma_start(
                    out=cin, in_=Cnt[b].rearrange("p j2 c l -> p (j2 c l)"))

            def gather():
                # ONE AllGather for both batches (cc ops here cost ~20us each,
                # so a second one would serialize behind the first)
                nc.gpsimd.collective_compute(
                    "AllGather", mybir.AluOpType.bypass,
                    replica_groups=[list(range(NCORES))],
                    ins=[adj_in.opt()], outs=[adj_gat.opt()])
                # gathered rows [kd, (b,j,c,l)]; load with 8x partition
                # replication so partition p=(dl,hb) holds row kd=dl
                FR2 = B * PATCH * C * PATCH  # 4096
                rep = bass.AP(tensor=adj_gat.tensor, offset=adj_gat.offset,
                              ap=[[FR2, DL], [0, POOL], [1, FR2]])
                nc.sync.dma_start(out=AdjR, in_=rep)

            def phase2(b):
                for h in range(NH):
                    cs = slice(h * CH, (h + 1) * CH)
                    for c4 in range(CH):
                        c = h * CH + c4
                        a_b = Ar[:, b, c].unsqueeze(1).unsqueeze(3) \
                            .broadcast_to([128, PATCH, G, PATCH])
                        r_b = AdjR[:, b, :, c].unsqueeze(2) \
                            .broadcast_to([128, PATCH, G, PATCH])
                        co_c = CO[:, c].rearrange("p (kh g kw) -> p kh g kw",
                                                  kh=PATCH, g=G)
                        nc.vector.tensor_mul(co_c, a_b, r_b)
                    ov2 = o2[b, h].rearrange("dl hb c k w -> (dl hb) (c k w)")
                    nc.sync.dma_start(
                        out=ov2, in_=CO[:, cs].rearrange("p c f -> p (c f)"))
                    nc.scalar.activation(
                        U2, CO[:, cs],
                        mybir.ActivationFunctionType.Square,
                        bias=1.0, scale=1.0)
                    # out1 in place over the resident x half
                    nc.vector.tensor_mul(X[b][:, cs], X[b][:, cs], U2)
                    ov1 = o1[b, h].rearrange("dl hb c k w -> (dl hb) (c k w)")
                    nc.scalar.dma_start(
                        out=ov1, in_=X[b][:, cs].rearrange("p c f -> p (c f)"))

            phase1(0)
            phase1(1)
            gather()
            phase2(0)
            phase2(1)

    nc.compile()
    return nc


def _host_inputs(x, attentions):
    """Build per-core input maps from full f32 inputs (fp16 cast + argmax fix)."""
    am = np.argmax(x, axis=1)              # [B,D,H,W], first-max == reference
    xh = x.astype(np.float16)
    mx = xh.max(axis=1, keepdims=True)
    notam = (np.arange(C)[None, :, None, None, None] != am[:, None])
    coll = (xh == mx) & notam
    if coll.any():
        u = xh.view(np.uint16)
        pos = xh > 0
        zero = xh == 0
        down = np.where(pos, u - np.uint16(1),
                        np.where(zero, np.uint16(0x8001), u + np.uint16(1)))
        xh = np.where(coll, down.view(np.float16), xh)

    att = attentions[..., 0].astype(np.float32) * np.float32(1.0 / 512.0)
    att_p = att.reshape(B, C, G, G, G)     # [b, c, dp, hp, wp]
    selm = np.zeros((128, 16), np.float16)
    p = np.arange(128)
    selm[p, (p // 64) * 8 + (p % 8)] = 1.0

    in_maps = []
    for core in range(NCORES):
        xc = xh[:, :, core * DL:(core + 1) * DL]        # [B, C, DL, H, W]
        xc = xc.reshape(B, NH, CH, DL, POOL, PATCH, W)
        xc = np.ascontiguousarray(xc.transpose(0, 1, 3, 4, 2, 5, 6))
        arep = np.ascontiguousarray(
            np.tile(att_p[:, :, core], (1, 1, DL, 1)).reshape(B, C, 128, G))
        in_maps.append({"xs": xc, "arep": arep, "sel": selm})
    return in_maps


def _unpack(o):
    """[B, NH, DL, POOL, CH, PATCH, W] packed core output -> [B,C,DL,H,W]."""
    o = np.asarray(o).transpose(0, 1, 4, 2, 3, 5, 6)
    return o.reshape(B, C, DL, H, W)


def kernel(x, attentions):
    x = np.asarray(x, dtype=np.float32)
    attentions = np.asarray(attentions, dtype=np.float32)

    if "nc" not in _CACHE:
        _CACHE["nc"] = _build_nc()
    nc = _CACHE["nc"]

    in_maps = _host_inputs(x, attentions)
    res = bass_utils.run_bass_kernel_spmd(nc, in_maps,
                                          core_ids=list(range(NCORES)))

    out1 = np.empty((B, C, D, H, W), np.float32)
    out2 = np.empty((B, C, D, H, W), np.float32)
    for core in range(NCORES):
        out1[:, :, core * DL:(core + 1) * DL] = _unpack(res.results[core]["o1"])
        out2[:, :, core * DL:(core + 1) * DL] = _unpack(res.results[core]["o2"])
    return out1, out2


# revision 27
# speedup vs baseline: 1.3629x; 1.3629x over previous
"""Trainium2 Bass kernel for the DPAAUser3D segment-reduce problem.

Computes, for x[B=2,C=8,D=H=W=128] and attentions[B,C,512,1]:
  onehot = one_hot(argmax_c x)                      (per-voxel channel argmax)
  adj    = avgpool_8x8x8(onehot)                    ([B,C,16,16,16], = counts/512)
  corr[b,c,D,H,W] = att[b,c,(D//16*8+H//16)*8+W//16] * adj[b,c,D%16,H%16,W%16]
  out1   = x * (1+corr)^2
  out2   = corr

Single-pass design: x is loaded once per core (data-parallel over D, 16
d-slices each) and stays resident in SBUF between the counting phase and the
output phase; all bulk IO is fp16 (~27 MiB per core HBM traffic).

Per batch, x is ONE SBUF mega-tile with partitions=(d_local, h//16) and
free=(c, h%16, w); the DRAM tensors use the matching packed layout so every
load/store is a fully-contiguous 2.1 MiB DMA. The one-hot is a single
[128,16k] fp16 mega-op; pooled counts go through contiguous-rhs PE matmuls
(0/1 selector contracts the d-blocks) + strided DVE reduces; one small 8 KiB
AllGather per batch distributes the count map (cc-core ops here cost ~2us/KB
+ peer skew, so small-and-early beats merged); out1 is computed in place
over the resident x with one mega-mul per C-half.

fp16 correctness: the host computes the f32 argmax (reference semantics) and
nudges any non-argmax channel that collides with the channel max in fp16 down
one ulp, so the device equality compare reproduces one_hot(argmax(f32 x))
exactly and the pooled counts are exact integers (<=512, exact in fp16). The
remaining error is fp16 rounding on x and on each output store (~1e-3 rel).
"""

import sys

import numpy as np

try:
    import concourse.bass as bass
except ImportError:  # fresh grading dir: concourse lives in the repo checkout
    for p in ("/opt/trn_rl_repo", "/root/.axon_site/_ro/trn_rl_repo"):
        if p not in sys.path:
            sys.path.insert(0, p)
    import concourse.bass as bass

import concourse.bacc as bacc
import concourse.mybir as mybir
import concourse.tile as tile
from concourse import bass_utils

B, C, D, H, W = 2, 8, 128, 128, 128
POOL = 8          # pooling block edge
PATCH = 16        # fold patch edge
G = D // PATCH    # 8 patches per spatial dim
NCORES = 8
DL = D // NCORES  # 16 d-slices per core
PD = DL // POOL   # 2 pooled kd-blocks per core
FS = C * PATCH * PATCH  # 2048: free size of the per-batch pooled-count map
NH = 2            # C split into halves of 4 channels for load/store overlap
CH = C // NH
CF = PATCH * W    # 2048: per-channel free size

F32 = mybir.dt.float32
F16 = mybir.dt.float16

_CACHE = {}


def _build_nc():
    nc = bacc.Bacc("TRN2", target_bir_lowering=False, debug=False,
                   num_devices=NCORES)

    # packed layout: [b, half, dl, hb, c4, k, w] (partition = (dl, hb))
    xs = nc.dram_tensor("xs", [B, NH, DL, POOL, CH, PATCH, W], F16,
                        kind="ExternalInput").ap()
    # arep[b,c,p,wp] = att[b,c, (core*8 + p%8)*8 + wp] / 512   (p = dl*8+hb)
    arep = nc.dram_tensor("arep", [B, C, 128, G], F32,
                          kind="ExternalInput").ap()
    # sel[p, (p//64)*8 + p%8] = 1: contracts the two 8-d-slice blocks
    sel = nc.dram_tensor("sel", [128, 16], F16, kind="ExternalInput").ap()
    o1 = nc.dram_tensor("o1", [B, NH, DL, POOL, CH, PATCH, W], F16,
                        kind="ExternalOutput").ap()
    o2 = nc.dram_tensor("o2", [B, NH, DL, POOL, CH, PATCH, W], F16,
                        kind="ExternalOutput").ap()

    with tile.TileContext(nc) as tc:
        with (
            tc.tile_pool(name="big", bufs=1) as big,
            tc.tile_pool(name="psum", bufs=2, space="PSUM") as pp,
            tc.tile_pool(name="dram", bufs=1, space="DRAM") as dram,
        ):
            Sel = big.tile([128, 16], F16, name="Sel")
            Ar = big.tile([128, B, C, G], F32, name="Ar")
            X = {b: big.tile([128, C, CF], F16, name=f"x{b}") for b in range(B)}
            EQ = big.tile([128, C, CF], F16, name="EQ")
            CO = big.tile([128, C, CF], F16, name="CO")
            U2 = big.tile([128, CH, CF], F16, name="U2")  # one half at a time
            m1 = [big.tile([128, CF], F16, name=f"m1_{i}") for i in range(4)]
            m2 = [big.tile([128, CF], F16, name=f"m2_{i}") for i in range(2)]
            M = big.tile([128, CF], F16, name="M")
            # AdjR[b][p=(dl,rep8), (j,c,l)] = counts[b, row dl] (8x part rep)
            AdjR = [big.tile([128, FS], F16, name=f"AdjR{b}") for b in range(B)]
            Cnt = [big.tile([16, 2, C, PATCH], F16, name=f"cnt{b}")
                   for b in range(B)]

            nc.sync.dma_start(out=Sel, in_=sel)
            for b in range(B):
                nc.sync.dma_start(out=Ar[:, b], in_=arep[b].transpose([1, 0, 2]))
            for b in range(B):
                for h in range(NH):
                    xv = xs[b, h].rearrange("dl hb c k w -> (dl hb) (c k w)")
                    nc.sync.dma_start(out=X[b][:, h * CH:(h + 1) * CH], in_=xv)

            adj_in = [dram.tile([PD, PATCH, C, PATCH], F16, name=f"adj_in{b}")
                      for b in range(B)]
            adj_gat = [dram.tile([NCORES, PD, PATCH, C, PATCH], F16,
                                 name=f"adj_gat{b}", addr_space="Shared")
                       for b in range(B)]

            def phase1(b):
                # channel max: pairwise tree (fp16 unit stride -> DVE 2x)
                for i in range(4):
                    nc.vector.tensor_max(m1[i], X[b][:, 2 * i], X[b][:, 2 * i + 1])
                nc.vector.tensor_max(m2[0], m1[0], m1[1])
                nc.vector.tensor_max(m2[1], m1[2], m1[3])
                nc.vector.tensor_max(M, m2[0], m2[1])
                # one-hot in a single mega-op
                nc.vector.tensor_tensor(
                    EQ, X[b], M.unsqueeze(1).broadcast_to([128, C, CF]),
                    op=mybir.AluOpType.is_equal)
                # pooled counts: per c, 4 contiguous [128,512] matmuls into two
                # k-half psums (Sel contracts d-blocks), 2 strided reduces fold
                # the k%8 / w%8 axes
                for c in range(C):
                    eqv = EQ[:, c].rearrange("p (kc x) -> p kc x", kc=4)
                    for half in range(2):
                        ps = pp.tile([16, 512], F32, name="ps",
                                     tag=f"ps{half}{c % 2}")
                        nc.tensor.matmul(ps, lhsT=Sel, rhs=eqv[:, 2 * half],
                                         start=True, stop=False)
                        nc.tensor.matmul(ps, lhsT=Sel, rhs=eqv[:, 2 * half + 1],
                                         start=False, stop=True)
                        # ps free = (k4, l=16, w8): sum the 8x8 h/w sub-block
                        pv = ps.rearrange("p (k4 l w8) -> p l k4 w8",
                                          k4=4, l=PATCH, w8=POOL)
                        # counts are integers <=512: exact in fp16
                        with nc.allow_low_precision(reason="integer counts"):
                            nc.vector.reduce_sum(Cnt[b][:, half, c], pv,
                                                 axis=mybir.AxisListType.XY)
                # ship counts: partition (pd,hb) + free (j2,c,l) lands on the
                # contiguous (j=hb*2+j2, c, l) rows of adj_in
                cin = adj_in[b].rearrange("pd (hb j2) c l -> (pd hb) (j2 c l)",
                                          hb=POOL)
                nc.gpsimd.dma_start(
                    out=cin, in_=Cnt[b].rearrange("p j2 c l -> p (j2 c l)"))
                nc.gpsimd.collective_compute(
                    "AllGather", mybir.AluOpType.bypass,
                    replica_groups=[list(range(NCORES))],
                    ins=[adj_in[b].opt()], outs=[adj_gat[b].opt()])

            def adjload(b):
                # gathered [kd, j, c, l]; load with 8x partition replication so
                # partition p=(dl,hb) holds row kd=dl (sync ring: idle by now)
                rep = bass.AP(tensor=adj_gat[b].tensor,
                              offset=adj_gat[b].offset,
                              ap=[[FS, DL], [0, POOL], [1, FS]])
                nc.sync.dma_start(out=AdjR[b], in_=rep)

            def phase2(b):
                for h in range(NH):
                    cs = slice(h * CH, (h + 1) * CH)
                    for c4 in range(CH):
                        c = h * CH + c4
                        a_b = Ar[:, b, c].unsqueeze(1).unsqueeze(3) \
                            .broadcast_to([128, PATCH, G, PATCH])
                        r_b = AdjR[b].rearrange("p (kh c kw) -> p kh c kw",
                                                c=C, kh=PATCH)[:, :, c] \
                            .unsqueeze(2).broadcast_to([128, PATCH, G, PATCH])
                        co_c = CO[:, c].rearrange("p (kh g kw) -> p kh g kw",
                                                  kh=PATCH, g=G)
                        nc.vector.tensor_mul(co_c, a_b, r_b)
                    ov2 = o2[b, h].rearrange("dl hb c k w -> (dl hb) (c k w)")
                    nc.sync.dma_start(
                        out=ov2, in_=CO[:, cs].rearrange("p c f -> p (c f)"))
                    nc.scalar.activation(
                        U2, CO[:, cs], mybir.ActivationFunctionType.Square,
                        bias=1.0, scale=1.0)
                    # out1 in place over the resident x half (mega-op hits 2x)
                    nc.vector.tensor_mul(X[b][:, cs], X[b][:, cs], U2)
                    ov1 = o1[b, h].rearrange("dl hb c k w -> (dl hb) (c k w)")
                    nc.scalar.dma_start(
                        out=ov1, in_=X[b][:, cs].rearrange("p c f -> p (c f)"))

            phase1(0)       # ... AG0 on gpsimd/cc-stream
            phase1(1)       # ... AG1 queued behind AG0 on the cc stream
            adjload(0)      # sync ring: waits AG0 only
            phase2(0)
            adjload(1)      # after b0's o2 pushes so they aren't queue-blocked
            phase2(1)

    nc.compile()
    return nc


def _host_inputs(x, attentions):
    """Build per-core input maps from full f32 inputs (fp16 cast + argmax fix)."""
    am = np.argmax(x, axis=1)              # [B,D,H,W], first-max == reference
    xh = x.astype(np.float16)
    mx = xh.max(axis=1, keepdims=True)
    notam = (np.arange(C)[None, :, None, None, None] != am[:, None])
    coll = (xh == mx) & notam
    if coll.any():
        u = xh.view(np.uint16)
        pos = xh > 0
        zero = xh == 0
        down = np.where(pos, u - np.uint16(1),
                        np.where(zero, np.uint16(0x8001), u + np.uint16(1)))
        xh = np.where(coll, down.view(np.float16), xh)

    att = attentions[..., 0].astype(np.float32) * np.float32(1.0 / 512.0)
    att_p = att.reshape(B, C, G, G, G)     # [b, c, dp, hp, wp]
    selm = np.zeros((128, 16), np.float16)
    p = np.arange(128)
    selm[p, (p // 64) * 8 + (p % 8)] = 1.0

    in_maps = []
    for core in range(NCORES):
        xc = xh[:, :, core * DL:(core + 1) * DL]        # [B, C, DL, H, W]
        xc = xc.reshape(B, NH, CH, DL, POOL, PATCH, W)
        xc = np.ascontiguousarray(xc.transpose(0, 1, 3, 4, 2, 5, 6))
        arep = np.ascontiguousarray(
            np.tile(att_p[:, :, core], (1, 1, DL, 1)).reshape(B, C, 128, G))
        in_maps.append({"xs": xc, "arep": arep, "sel": selm})
    return in_maps


def _unpack(o):
    """[B, NH, DL, POOL, CH, PATCH, W] packed core output -> [B,C,DL,H,W]."""
    o = np.asarray(o).transpose(0, 1, 4, 2, 3, 5, 6)
    return o.reshape(B, C, DL, H, W)


def kernel(x, attentions):
    x = np.asarray(x, dtype=np.float32)
    attentions = np.asarray(attentions, dtype=np.float32)

    if "nc" not in _CACHE:
        _CACHE["nc"] = _build_nc()
    nc = _CACHE["nc"]

    in_maps = _host_inputs(x, attentions)
    res = bass_utils.run_bass_kernel_spmd(nc, in_maps,
                                          core_ids=list(range(NCORES)))

    out1 = np.empty((B, C, D, H, W), np.float32)
    out2 = np.empty((B, C, D, H, W), np.float32)
    for core in range(NCORES):
        out1[:, :, core * DL:(core + 1) * DL] = _unpack(res.results[core]["o1"])
        out2[:, :, core * DL:(core + 1) * DL] = _unpack(res.results[core]["o2"])
    return out1, out2
